# revision 21
# baseline (speedup 1.0000x reference)
"""Trainium2 Bass kernel for MinibatchDiscrimination (symmetric scheme, v2).

Reference computation:
    M = (x @ T.reshape(A, B*C)).reshape(N, B, C)        x:[2048,512] T:[512,64,16]
    O[i, b] = sum_j exp(-sum_c |M[j,b,c] - M[i,b,c]|)    O:[2048,64]

K(i,j) is symmetric; each unordered pair is computed once and credited to both
row-sums.  Core c owns rows R_c = [256c, 256c+256), processed as 128 pairs q
(rows 2q, 2q+1).  Coverage per pair (local window cols):

  - self block  [0, 256):   pairs q < 64 compute all 256 cols; pairs q >= 64
    compute only [128, 256).  The missing orientation (rows [128,256) x cols
    [0,128)) is recovered from the q<64 exps via a column-sum over local cols
    [128, 256) (csSelf).  Intra-block pairs otherwise appear in both
    orientations, so no other self credit is needed.
  - next 3 blocks [256, 1024): full; column-sums credit the partner rows.
  - antipodal block (c+4): 128 of its 256 cols at FULL weight: pairs q<64 get
    one half, q>=64 the complementary half; cores 4-7 use the swapped halves
    (host swaps the two 128-col groups in their input) so that every
    cross-antipodal pair is computed exactly once.  Column-sums (csAntiLo for
    q<64, csAntiHi for q>=64) credit the partner rows.

So window W_q = 1152 for q < 64 and 1024 for q >= 64.

Engine split per pair (production of absdiff slabs A[(s,ii)] = |MT - MT[:,i]|,
bc on partitions as 8 t-blocks x 128, window on free dim):
  - ACT: s=0,1 / ii=0 slabs in fp8 (feed DoubleRow matmuls), (2,0,0) bf16 and
    the tail [SPLIT:] of (2,0,1); also the exp and its accum_out (row-sums).
  - DVE: everything else in bf16 via 2 passes (tensor_scalar add-neg, then
    joint u16 sign-clear).
  - PE: c-contraction into psum [2i x 64b, W], + column-sum matmuls.
MT is kept twice: MT_lo (self+mid+antiLo, 1152) and MT_hi (self+mid+antiHi)
copied via an idle SBUF->SBUF DMA so every slab read is contiguous.

Sync discipline: every engine instruction carries at most ONE sem wait
(walrus limit); cross-engine deps are subsumed through per-pair absorber
instructions (baseline-proven pattern).
"""

import numpy as np
import ml_dtypes

N, A_DIM, B, C = 2048, 512, 64, 16
BC = B * C
N_CORES = 8
SHARD = N // N_CORES          # 256
PAIRS = SHARD // 2            # 128
HALF = PAIRS // 2             # 64 pairs per q-half
W = 1152                      # self + mid + one antipodal half
WIN = 1280                    # prologue j window (both antipodal halves)

XT_W = WIN
TM_OFF = XT_W                 # 1280
SB_OFF = TM_OFF + BC          # 2304
SELDR_OFF = SB_OFF + 120      # 2424
SELCS_OFF = SELDR_OFF + 128   # 2552
TOT_W = SELCS_OFF + 64        # 2616

# ACT-produced slabs: (s,ii) in FP8_G are fp8 (both kt); (2,0,0) is ACT bf16;
# (2,0,1) is split: DVE does [off_q, SPLIT), ACT does [SPLIT, 1152).
FP8_G = ((0, 0), (1, 0))
SPLIT_LO = 796                # q < 64  (window [0, 1152))
SPLIT_HI = 820                # q >= 64 (window [128, 1152))

CHUNKS_LO = ((0, 512), (512, 512), (1024, 128))
CHUNKS_HI = ((128, 384), (512, 512), (1024, 128))

# cs psum layout (2 banks):
#   csA [128,512]  <- eprev[:, 256:768)                       (all pairs)
#   csB [128,512]: [0:256)   <- eprev[:, 768:1024)            (all pairs)
#                  [256:384) <- eprev[:, 128:256)  q<64 csSelf, then reused
#                               as csAntiHi (q>=64, eprev[:, 1024:1152))
#                  [384:512) <- eprev[:, 1024:1152) q<64 csAntiLo
# csSelf is copied to OutS right after pair 64's cs matmuls, then the slot is
# restarted (start=True) as csAntiHi.
CS_OUT = 768 + 128 + 128 + 128  # common 768 | self 128 | antiLo 128 | antiHi 128
OUT_W = PAIRS + CS_OUT          # 1280

_CACHE = {}


def _build_nc(npairs=PAIRS):
    from contextlib import ExitStack
    import concourse.bass as bass
    import concourse.mybir as mybir
    from concourse.tile import TileContext, add_dep_helper

    f32 = mybir.dt.float32
    bf16 = mybir.dt.bfloat16
    fp8 = mybir.dt.float8e4
    u16 = mybir.dt.uint16
    Abs = mybir.ActivationFunctionType.Abs
    Exp = mybir.ActivationFunctionType.Exp
    add_op = mybir.AluOpType.add
    band_op = mybir.AluOpType.bitwise_and
    DR = mybir.MatmulPerfMode.DoubleRow

    nc = bass.Bass("TRN2", target_bir_lowering=False, debug=False)
    XTm = nc.dram_tensor("XTm", [A_DIM, TOT_W], bf16, kind="ExternalInput").ap()
    OUT = nc.dram_tensor("OUT", [128, OUT_W], f32, kind="ExternalOutput").ap()

    with TileContext(nc) as tc, ExitStack() as ctx:
        singles = ctx.enter_context(tc.tile_pool(name="singles", bufs=1))

        XAll = singles.tile([128, 4, TOT_W], bf16, name="XAll", tag="XAll")
        nc.sync.dma_start(out=XAll, in_=XTm.rearrange("(k p) n -> p k n", p=128))

        Sbig = singles.tile([128, 120], bf16, name="Sbig", tag="Sbig")
        nc.scalar.copy(Sbig, XAll[:, 0, SB_OFF:SB_OFF + 120])
        SelDR = singles.tile([128, 2, 128], fp8, name="SelDR", tag="SelDR")
        nc.scalar.copy(SelDR.rearrange("p a b -> p (a b)"),
                       XAll[:, 0, SELDR_OFF:SELDR_OFF + 128].bitcast(fp8))
        SelCS = singles.tile([128, 64], bf16, name="SelCS", tag="SelCS")
        nc.scalar.copy(SelCS, XAll[:, 0, SELCS_OFF:SELCS_OFF + 64])

        MT = singles.tile([128, 8, WIN], bf16, name="MT", tag="MT")
        MTlo = MT[:, :, 0:W]
        MThi = singles.tile([128, 8, W], bf16, name="MThi", tag="MThi")
        negMTi = singles.tile([128, 8, SHARD], f32, name="negMTi", tag="negMTi")
        OutS = singles.tile([128, OUT_W], f32, name="OutS", tag="OutS")
        O_sb = OutS[:, 0:PAIRS]
        csS = OutS[:, PAIRS:]
        mask16 = singles.tile([128, 1], u16, name="mask16", tag="mask16")
        nc.vector.memset(mask16, 0x7FFF)
        nc.vector.memset(OutS, 0.0)
        junkD = singles.tile([1, PAIRS], f32, name="junkD", tag="junkD")
        junkA = singles.tile([1, PAIRS], f32, name="junkA", tag="junkA")
        junkE = singles.tile([1, PAIRS], f32, name="junkE", tag="junkE")
        junkX = singles.tile([1, PAIRS], f32, name="junkX", tag="junkX")

        with tc.tile_pool(name="cspsum", bufs=1, space="PSUM") as cspool, \
             tc.tile_pool(name="mpsum", bufs=2, space="PSUM") as mpsum, \
             tc.tile_pool(name="apool", bufs=2) as apool, \
             tc.tile_pool(name="epool", bufs=2) as epool:
            csA = cspool.tile([128, 512], f32, name="csA", tag="csA")
            csB = cspool.tile([128, 512], f32, name="csB", tag="csB")

            # ---- prologue: MT = (x @ T)^T in bf16 (psum f32); cols [0:1152)
            # -> MTlo, [1152:1280) -> MThi[..., 1024:1152).  psum->SBUF copies
            # alternate ACT/DVE.  Then one SBUF->SBUF DMA dups [0:1024).
            last_pro_dve = None
            ci = 0
            for m in range(8):
                for (clo, cw) in ((0, 512), (512, 512), (1024, 256)):
                    ps = mpsum.tile([128, 1536], f32, name="ps", tag="ps")
                    for k in range(4):
                        nc.tensor.matmul(
                            ps[:, :cw],
                            XAll[:, k, TM_OFF + 128 * m:TM_OFF + 128 * (m + 1)],
                            XAll[:, k, clo:clo + cw],
                            start=(k == 0), stop=(k == 3),
                        )
                    # all MT copies on ACT: single-engine MT writers keep every
                    # consumer at one sem wait regardless of scheduler sinking
                    nc.scalar.copy(MT[:, m, clo:clo + cw], ps[:, :cw])
                    ci += 1
                last_pro_act = nc.scalar.mul(negMTi[:, m, :], MTlo[:, m, 0:SHARD], -1.0)

            # idle-DMA duplication of self+mid into MThi.  absorber_pro makes
            # the ACT clock dominate the DVE prologue copies so the DMACopy
            # wait-pruning below (keep ACT only) stays correct.
            dma_mt1 = nc.sync.dma_start(out=MThi[:, :, 0:1024],
                                        in_=MT[:, :, 0:1024])
            dma_mt = nc.sync.dma_start(out=MThi[:, :, 1024:1152],
                                       in_=MT[:, :, 1152:1280])

            prev_mm_last = {}
            prev_dve_last = {}
            prev_exp = {}
            for q in range(npairs):
                lo = q < HALF
                MTq = MTlo if lo else MThi
                off = 0 if lo else 128
                split = SPLIT_LO if lo else SPLIT_HI
                chunks = CHUNKS_LO if lo else CHUNKS_HI
                cols = (2 * q, 2 * q + 1)

                # --- A tiles
                A = {}
                for s in range(4):
                    for ii in range(2):
                        dt_s = fp8 if (s, ii) in FP8_G else bf16
                        A[(s, ii)] = apool.tile([128, 2, W], dt_s,
                                                name=f"A{s}_{ii}", tag=f"A{s}_{ii}",
                                                bufs=2)

                # --- ACT absorbers
                act_markers = []
                if q >= 2:
                    absorber_pa = nc.scalar.copy(junkA[:, q:q + 1],
                                                 negMTi[0:1, 1, q:q + 1])
                    add_dep_helper(absorber_pa.ins, prev_mm_last[q - 2].ins,
                                   sync=True, reason="advance ACT PE-clock")
                    absorber_pd = nc.scalar.copy(junkE[:, q:q + 1],
                                                 negMTi[0:1, 0, q:q + 1])
                    add_dep_helper(absorber_pd.ins, prev_dve_last[q - 2].ins,
                                   sync=True, reason="advance ACT DVE-clock")
                    act_markers += [absorber_pa, absorber_pd]
                elif q == 0 and last_pro_dve is not None:
                    absorber_p0 = nc.scalar.copy(junkE[:, 0:1],
                                                 negMTi[0:1, 0, 0:1])
                    add_dep_helper(absorber_p0.ins, last_pro_dve.ins, sync=True,
                                   reason="ACT sees DVE prologue MT copies")
                    act_markers.append(absorber_p0)
                if q == HALF:
                    # MThi ready (DMA) before first hi-half consumer
                    absorber_mt = nc.scalar.copy(junkA[:, 1:2], negMTi[0:1, 2, 0:1])
                    add_dep_helper(absorber_mt.ins, dma_mt.ins, sync=True,
                                   reason="ACT waits MThi dup DMA")
                    act_markers.append(absorber_mt)

                # --- ACT absdiff slabs: fp8 groups then bf16 (2,0,0) + tail
                act_slabs = [(0, 0, 0), (0, 0, 1), (1, 0, 0), (1, 0, 1), (2, 0, 0)]
                for (s, ii, kt) in act_slabs:
                    t = 2 * s + kt
                    act_i = nc.scalar.activation(
                        A[(s, ii)][:, kt, off:W], MTq[:, t, off:W], Abs,
                        bias=negMTi[:, t, cols[ii]:cols[ii] + 1])
                    for mk in act_markers:
                        add_dep_helper(act_i.ins, mk.ins, sync=False,
                                       reason="order absdiff after ACT absorbers")
                t = 2 * 2 + 1
                act_i = nc.scalar.activation(
                    A[(2, 0)][:, 1, split:W], MTq[:, t, split:W], Abs,
                    bias=negMTi[:, t, cols[0]:cols[0] + 1])
                for mk in act_markers:
                    add_dep_helper(act_i.ins, mk.ins, sync=False,
                                   reason="order absdiff after ACT absorbers")

                # --- DVE absorbers
                dve_markers = []
                if q != 1:
                    absorber_dv = nc.vector.tensor_copy(junkD[:, q:q + 1],
                                                        negMTi[0:1, 0, q:q + 1])
                    dep = last_pro_act if q == 0 else prev_mm_last[q - 2]
                    add_dep_helper(absorber_dv.ins, dep.ins, sync=True,
                                   reason="advance DVE clock (prologue/PE WAR)")
                    dve_markers.append(absorber_dv)
                if q == HALF:
                    absorber_mtd = nc.vector.tensor_copy(junkD[:, 0:1],
                                                         negMTi[0:1, 3, 0:1])
                    add_dep_helper(absorber_mtd.ins, dma_mt.ins, sync=True,
                                   reason="DVE waits MThi dup DMA")
                    dve_markers.append(absorber_mtd)

                # --- DVE absdiff subs; the ACT-shared (2,0) tile goes LAST so
                # its cross-engine tile wait is the instruction's only one.
                dve_subs = []
                for s in (3,):
                    for kt in range(2):
                        dve_subs.append(((s, 0), kt, off, W))
                for s in range(4):
                    for kt in range(2):
                        dve_subs.append(((s, 1), kt, off, W))
                dve_subs.append(((2, 0), 1, off, split))
                first_sub = True
                for (g, kt, c0, c1) in dve_subs:
                    t = 2 * g[0] + kt
                    ts = nc.vector.tensor_scalar(
                        A[g][:, kt, c0:c1], MTq[:, t, c0:c1],
                        negMTi[:, t, cols[g[1]]:cols[g[1]] + 1], None, op0=add_op)
                    for mk in dve_markers:
                        # sync on the first sub propagates the absorber's
                        # vector clock (a nosync edge only orders, leaving a
                        # second redundant sem wait at q==0)
                        add_dep_helper(ts.ins, mk.ins, sync=first_sub,
                                       reason="order after DVE absorbers")
                    first_sub = False

                # --- DVE sign-clear passes: joints, then the (2,0) partial
                for g in ((3, 0), (0, 1), (1, 1), (2, 1), (3, 1)):
                    view = A[g][:, :, off:W].bitcast(u16)
                    nc.vector.tensor_scalar(view, view, mask16, None, op0=band_op)
                view = A[(2, 0)][:, 1, off:split].bitcast(u16)
                dve_last = nc.vector.tensor_scalar(view, view, mask16, None,
                                                   op0=band_op)

                # --- PE: c-contraction into psum [2i x 64b, W]
                ps = mpsum.tile([128, 1536], f32, name="ps", tag="ps")
                if q >= 2:
                    dmy = nc.tensor.matmul(ps[0:32, 0:1], Sbig[:, 0:32],
                                           Sbig[:, 1:2], start=True, stop=True,
                                           skip_group_check=True)
                    add_dep_helper(dmy.ins, prev_mm_last[q - 2].ins,
                                   sync=True, reason="advance PE own-clock for psum WAW")
                mm1 = None
                mm = None
                first_group = True
                for ii in range(2):
                    for (clo, cw) in chunks:
                        out = ps[64 * ii:64 * (ii + 1), clo:clo + cw]
                        if ii == 0:
                            mm = nc.tensor.matmul(
                                out, SelDR[:, :, 64:128],
                                A[(0, 0)][:, :, clo:clo + cw],
                                start=True, stop=False, perf_mode=DR)
                            if mm1 is None:
                                mm1 = mm
                                add_dep_helper(mm1.ins, act_i.ins, sync=True,
                                               reason="PE waits last ACT slab")
                            nc.tensor.matmul(
                                out, SelDR[:, :, 48:112],
                                A[(1, 0)][:, :, clo:clo + cw],
                                start=False, stop=False, perf_mode=DR)
                            for s in (2, 3):
                                for kt in range(2):
                                    t = 2 * s + kt
                                    mm = nc.tensor.matmul(
                                        out, Sbig[:, 56 - 8 * t:120 - 8 * t],
                                        A[(s, 0)][:, kt, clo:clo + cw],
                                        start=False, stop=(s == 3 and kt == 1))
                        else:
                            for s in range(4):
                                for kt in range(2):
                                    t = 2 * s + kt
                                    mm = nc.tensor.matmul(
                                        out, Sbig[:, 56 - 8 * t:120 - 8 * t],
                                        A[(s, 1)][:, kt, clo:clo + cw],
                                        start=(s == 0 and kt == 0),
                                        stop=(s == 3 and kt == 1))
                        if first_group and q >= 1:
                            # column-sums for pair q-1 (expS ready)
                            plo = (q - 1) < HALF
                            eprev = prev_exp[q - 1][1]
                            nc.tensor.matmul(csA[0:64], SelCS, eprev[:, 256:768],
                                             start=(q == 1), stop=(q == npairs),
                                             skip_group_check=True)
                            nc.tensor.matmul(csB[0:64, 0:256], SelCS,
                                             eprev[:, 768:1024],
                                             start=(q == 1), stop=(q == npairs),
                                             skip_group_check=True)
                            if plo:
                                nc.tensor.matmul(csB[0:64, 256:384], SelCS,
                                                 eprev[:, 128:256],
                                                 start=(q == 1), stop=(q == HALF),
                                                 skip_group_check=True)
                                nc.tensor.matmul(csB[0:64, 384:512], SelCS,
                                                 eprev[:, 1024:1152],
                                                 start=(q == 1), stop=(q == HALF),
                                                 skip_group_check=True)
                            else:
                                nc.tensor.matmul(csB[0:64, 256:384], SelCS,
                                                 eprev[:, 1024:1152],
                                                 start=(q == HALF + 1),
                                                 stop=(q == npairs),
                                                 skip_group_check=True)
                        first_group = False

                # --- csSelf evacuation right after its last accumulation
                if q == HALF:
                    cse = nc.scalar.copy(csS[0:64, 768:896], csB[0:64, 256:384])

                # --- exp -> SBUF bf16; accum_out writes the rowsum column
                if q >= 1:
                    absorber_ex = nc.scalar.copy(junkX[:, q:q + 1],
                                                 negMTi[0:1, 2, q:q + 1])
                    add_dep_helper(absorber_ex.ins, prev_exp[q - 1][0].ins,
                                   sync=True, reason="advance ACT own accum clock")
                    add_dep_helper(absorber_ex.ins, act_i.ins, sync=False,
                                   reason="pin after this pair's absdiffs")
                expS = epool.tile([128, W], bf16, name="expS", tag="expS", bufs=2)
                exp_i = nc.scalar.activation(expS[:, off:W], ps[:, off:W], Exp,
                                             scale=-1.0,
                                             accum_out=O_sb[:, q:q + 1])
                prev_exp[q] = (exp_i, expS)
                prev_mm_last[q] = mm
                prev_dve_last[q] = dve_last

            # column-sums of the last pair
            elast = prev_exp[npairs - 1][1]
            nc.tensor.matmul(csA[0:64], SelCS, elast[:, 256:768],
                             start=(npairs == 1), stop=True, skip_group_check=True)
            nc.tensor.matmul(csB[0:64, 0:256], SelCS, elast[:, 768:1024],
                             start=(npairs == 1), stop=True, skip_group_check=True)
            if npairs - 1 < HALF:
                nc.tensor.matmul(csB[0:64, 256:384], SelCS, elast[:, 128:256],
                                 start=(npairs == 1), stop=True,
                                 skip_group_check=True)
                nc.tensor.matmul(csB[0:64, 384:512], SelCS, elast[:, 1024:1152],
                                 start=(npairs == 1), stop=True,
                                 skip_group_check=True)
            else:
                nc.tensor.matmul(csB[0:64, 256:384], SelCS, elast[:, 1024:1152],
                                 start=(npairs == HALF + 1), stop=True,
                                 skip_group_check=True)

            nc.scalar.copy(csS[0:64, 0:512], csA[0:64])
            nc.scalar.copy(csS[0:64, 512:768], csB[0:64, 0:256])
            if npairs > HALF:
                nc.scalar.copy(csS[0:64, 1024:1152], csB[0:64, 256:384])
            else:
                nc.scalar.copy(csS[0:64, 768:896], csB[0:64, 256:384])
            nc.scalar.copy(csS[0:64, 896:1024], csB[0:64, 384:512])
            dma_o = nc.sync.dma_start(out=OUT, in_=OutS)

    # Kernel-tail Drain wait pruning (see baseline): every proc's completion is
    # transitively dominated by the output DMA; keep only its queue sem.
    out_upd = {u.ant_name for u in dma_o.ins.sync_info.on_update}
    for f in nc.m.functions:
        for bb in f.blocks:
            for ins in bb.instructions:
                si = getattr(ins, 'sync_info', None)
                if si is None or len(si.on_wait) <= 1:
                    continue
                if ins.opcode == 'Drain':
                    kept = [w for w in si.on_wait if w.ant_name in out_upd]
                    assert kept, f"drain {ins.name} has no output-queue wait"
                    si.on_wait = kept
                elif ins.opcode == 'DMACopy':
                    kept = [w for w in si.on_wait if 'Activation' in w.ant_name]
                    assert kept, f"dma {ins.name} has no ACT wait"
                    si.on_wait = kept
    return nc


def _host_inputs(x, T):
    bf = ml_dtypes.bfloat16
    xT = np.ascontiguousarray(np.asarray(x, dtype=np.float32).T).astype(bf)   # [512, 2048]
    Tm = np.asarray(T, dtype=np.float32).reshape(A_DIM, BC).astype(bf)

    consts = np.zeros((A_DIM, TOT_W - SB_OFF), dtype=bf)
    # Sbig: [128, 120] with Sb[p, 56 + p//16] = 1 (windows for bf16 matmuls)
    for p in range(128):
        consts[p, 56 + p // 16] = 1
    # SelDR: [128, 2, 128] fp8, sel[p, kt, 64 + 8kt + p//16] = 1; windows
    # [64:128) select s=0, [48:112) select s=1.
    sel = np.zeros((128, 2, 128), dtype=ml_dtypes.float8_e4m3)
    for p in range(128):
        for kt in range(2):
            sel[p, kt, 64 + 8 * kt + p // 16] = 1
    selb = sel.reshape(128, 256).view(np.uint16).view(bf)                      # [128, 128]
    consts[:128, SELDR_OFF - SB_OFF:SELCS_OFF - SB_OFF] = selb
    # SelCS: [128, 64] with selcs[p, p % 64] = 1
    for p in range(128):
        consts[p, SELCS_OFF - SB_OFF + (p % 64)] = 1

    in_maps = []
    for c in range(N_CORES):
        xT_rot = np.roll(xT, -c * SHARD, axis=1)[:, :WIN].copy()
        if c >= 4:
            anti = xT_rot[:, 1024:1280].copy()
            xT_rot[:, 1024:1152] = anti[:, 128:256]
            xT_rot[:, 1152:1280] = anti[:, 0:128]
        XTmc = np.ascontiguousarray(
            np.concatenate([xT_rot, Tm, consts], axis=1))
        in_maps.append({"XTm": XTmc})
    return in_maps


def run(x, T, npairs=PAIRS, trace=False):
    from concourse.bass_utils import run_bass_kernel_spmd

    nc = _CACHE.get(npairs)
    if nc is None:
        nc = _build_nc(npairs)
        _CACHE[npairs] = nc
    in_maps = _host_inputs(x, T)
    res = run_bass_kernel_spmd(nc, in_maps, list(range(N_CORES)), trace=trace)
    parts = []
    for c in range(N_CORES):
        out = np.asarray(res.results[c]["OUT"], dtype=np.float32)
        r = out[:, :PAIRS]                                   # [64ii+b, q]
        parts.append(r.reshape(2, B, PAIRS).transpose(2, 0, 1).reshape(SHARD, B))
    O = np.concatenate(parts, axis=0)
    for c in range(N_CORES):
        cs = np.asarray(res.results[c]["OUT"], dtype=np.float32)[0:B, PAIRS:]
        # common: local cols [256, 1024) -> global rows (256c + L) % N
        idx = (np.arange(256, 1024) + SHARD * c) % N
        O[idx] += cs[:, 0:768].T
        # csSelf: local cols [128, 256) -> own shard rows
        idx = SHARD * c + np.arange(128, 256)
        O[idx] += cs[:, 768:896].T
        # antipodal: partner block rows
        partner = (c + 4) % N_CORES
        if c < 4:
            lo_rows = SHARD * partner + np.arange(0, 128)
            hi_rows = SHARD * partner + np.arange(128, 256)
        else:
            lo_rows = SHARD * partner + np.arange(128, 256)
            hi_rows = SHARD * partner + np.arange(0, 128)
        O[lo_rows] += cs[:, 896:1024].T
        O[hi_rows] += cs[:, 1024:1152].T
    return O, res


def kernel(x, T):
    O, _ = run(x, T)
    return O


# revision 26
# speedup vs baseline: 1.0225x; 1.0225x over previous
"""Trainium2 Bass kernel for MinibatchDiscrimination (symmetric scheme, v2).

Reference computation:
    M = (x @ T.reshape(A, B*C)).reshape(N, B, C)        x:[2048,512] T:[512,64,16]
    O[i, b] = sum_j exp(-sum_c |M[j,b,c] - M[i,b,c]|)    O:[2048,64]

K(i,j) is symmetric; each unordered pair is computed once and credited to both
row-sums.  Core c owns rows R_c = [256c, 256c+256), processed as 128 pairs q
(rows 2q, 2q+1).  Coverage per pair (local window cols):

  - self block  [0, 256):   pairs q < 64 compute all 256 cols; pairs q >= 64
    compute only [128, 256).  The missing orientation (rows [128,256) x cols
    [0,128)) is recovered from the q<64 exps via a column-sum over local cols
    [128, 256) (csSelf).  Intra-block pairs otherwise appear in both
    orientations, so no other self credit is needed.
  - next 3 blocks [256, 1024): full; column-sums credit the partner rows.
  - antipodal block (c+4): 128 of its 256 cols at FULL weight: pairs q<64 get
    one half, q>=64 the complementary half; cores 4-7 use the swapped halves
    (host swaps the two 128-col groups in their input) so that every
    cross-antipodal pair is computed exactly once.  Column-sums (csAntiLo for
    q<64, csAntiHi for q>=64) credit the partner rows.

So window W_q = 1152 for q < 64 and 1024 for q >= 64.

Engine split per pair (production of absdiff slabs A[(s,ii)] = |MT - MT[:,i]|,
bc on partitions as 8 t-blocks x 128, window on free dim):
  - ACT: s=0,1 / ii=0 slabs in fp8 (feed DoubleRow matmuls), (2,0,0) bf16 and
    the tail [SPLIT:] of (2,0,1); also the exp and its accum_out (row-sums).
  - DVE: everything else in bf16 via 2 passes (tensor_scalar add-neg, then
    joint u16 sign-clear).
  - PE: c-contraction into psum [2i x 64b, W], + column-sum matmuls.
MT is kept twice: MT_lo (self+mid+antiLo, 1152) and MT_hi (self+mid+antiHi)
copied via an idle SBUF->SBUF DMA so every slab read is contiguous.

Sync discipline: every engine instruction carries at most ONE sem wait
(walrus limit); cross-engine deps are subsumed through per-pair absorber
instructions (baseline-proven pattern).
"""

import numpy as np
import ml_dtypes

N, A_DIM, B, C = 2048, 512, 64, 16
BC = B * C
N_CORES = 8
SHARD = N // N_CORES          # 256
PAIRS = SHARD // 2            # 128
HALF = PAIRS // 2             # 64 pairs per q-half
W = 1152                      # self + mid + one antipodal half
WIN = 1280                    # prologue j window (both antipodal halves)

XT_W = WIN
TM_OFF = XT_W                 # 1280
SB_OFF = TM_OFF + BC          # 2304
SELDR_OFF = SB_OFF + 120      # 2424
SELCS_OFF = SELDR_OFF + 128   # 2552
TOT_W = SELCS_OFF + 64        # 2616

# ACT-produced slabs: (s,ii) in FP8_G are fp8 (both kt); (2,0,0) is ACT bf16;
# (2,0,1) is split: DVE does [off_q, SPLIT), ACT does [SPLIT, 1152).
FP8_G = ((0, 0), (1, 0))
SPLIT_LO = 796                # q < 64  (window [0, 1152))
SPLIT_HI = 820                # q >= 64 (window [128, 1152))

CHUNKS_LO = ((0, 512), (512, 512), (1024, 128))
CHUNKS_HI = ((128, 384), (512, 512), (1024, 128))

# cs psum layout (2 banks):
#   csA [128,512]  <- eprev[:, 256:768)                       (all pairs)
#   csB [128,512]: [0:256)   <- eprev[:, 768:1024)            (all pairs)
#                  [256:384) <- eprev[:, 128:256)  q<64 csSelf, then reused
#                               as csAntiHi (q>=64, eprev[:, 1024:1152))
#                  [384:512) <- eprev[:, 1024:1152) q<64 csAntiLo
# csSelf is copied to OutS right after pair 64's cs matmuls, then the slot is
# restarted (start=True) as csAntiHi.
CS_OUT = 768 + 128 + 128 + 128  # common 768 | self 128 | antiLo 128 | antiHi 128
OUT_W = PAIRS + CS_OUT          # 1280

_CACHE = {}


def _build_nc(npairs=PAIRS):
    from contextlib import ExitStack
    import concourse.bass as bass
    import concourse.mybir as mybir
    from concourse.tile import TileContext, add_dep_helper

    f32 = mybir.dt.float32
    bf16 = mybir.dt.bfloat16
    fp8 = mybir.dt.float8e4
    u16 = mybir.dt.uint16
    Abs = mybir.ActivationFunctionType.Abs
    Exp = mybir.ActivationFunctionType.Exp
    add_op = mybir.AluOpType.add
    band_op = mybir.AluOpType.bitwise_and
    DR = mybir.MatmulPerfMode.DoubleRow

    nc = bass.Bass("TRN2", target_bir_lowering=False, debug=False)
    XTm = nc.dram_tensor("XTm", [A_DIM, TOT_W], bf16, kind="ExternalInput").ap()
    OUT = nc.dram_tensor("OUT", [128, OUT_W], f32, kind="ExternalOutput").ap()

    with TileContext(nc) as tc, ExitStack() as ctx:
        singles = ctx.enter_context(tc.tile_pool(name="singles", bufs=1))

        XAll = singles.tile([128, 4, TOT_W], bf16, name="XAll", tag="XAll")
        nc.sync.dma_start(out=XAll, in_=XTm.rearrange("(k p) n -> p k n", p=128))

        Sbig = singles.tile([128, 120], bf16, name="Sbig", tag="Sbig")
        nc.scalar.copy(Sbig, XAll[:, 0, SB_OFF:SB_OFF + 120])
        SelDR = singles.tile([128, 2, 128], fp8, name="SelDR", tag="SelDR")
        nc.scalar.copy(SelDR.rearrange("p a b -> p (a b)"),
                       XAll[:, 0, SELDR_OFF:SELDR_OFF + 128].bitcast(fp8))
        SelCS = singles.tile([128, 64], bf16, name="SelCS", tag="SelCS")
        nc.scalar.copy(SelCS, XAll[:, 0, SELCS_OFF:SELCS_OFF + 64])

        MT = singles.tile([128, 8, WIN], bf16, name="MT", tag="MT")
        MTlo = MT[:, :, 0:W]
        MThi = singles.tile([128, 8, W], bf16, name="MThi", tag="MThi")
        negMTi = singles.tile([128, 8, SHARD], f32, name="negMTi", tag="negMTi")
        OutS = singles.tile([128, OUT_W], f32, name="OutS", tag="OutS")
        O_sb = OutS[:, 0:PAIRS]
        csS = OutS[:, PAIRS:]
        mask16 = singles.tile([128, 1], u16, name="mask16", tag="mask16")
        nc.vector.memset(mask16, 0x7FFF)
        junkD = singles.tile([1, PAIRS], f32, name="junkD", tag="junkD")
        junkA = singles.tile([1, PAIRS], f32, name="junkA", tag="junkA")
        junkE = singles.tile([1, PAIRS], f32, name="junkE", tag="junkE")
        junkX = singles.tile([1, PAIRS], f32, name="junkX", tag="junkX")

        with tc.tile_pool(name="cspsum", bufs=1, space="PSUM") as cspool, \
             tc.tile_pool(name="mpsum", bufs=2, space="PSUM") as mpsum, \
             tc.tile_pool(name="apool", bufs=2) as apool, \
             tc.tile_pool(name="epool", bufs=2) as epool:
            csA = cspool.tile([128, 512], f32, name="csA", tag="csA")
            csB = cspool.tile([128, 512], f32, name="csB", tag="csB")

            # ---- prologue: MT = (x @ T)^T in bf16 (psum f32); cols [0:1152)
            # -> MTlo, [1152:1280) -> MThi[..., 1024:1152).  psum->SBUF copies
            # alternate ACT/DVE.  Then one SBUF->SBUF DMA dups [0:1024).
            last_pro_dve = None
            ci = 0
            for m in range(8):
                for (clo, cw) in ((0, 512), (512, 512), (1024, 256)):
                    ps = mpsum.tile([128, 1536], f32, name="ps", tag="ps")
                    for k in range(4):
                        nc.tensor.matmul(
                            ps[:, :cw],
                            XAll[:, k, TM_OFF + 128 * m:TM_OFF + 128 * (m + 1)],
                            XAll[:, k, clo:clo + cw],
                            start=(k == 0), stop=(k == 3),
                        )
                    # all MT copies on ACT: single-engine MT writers keep every
                    # consumer at one sem wait regardless of scheduler sinking
                    nc.scalar.copy(MT[:, m, clo:clo + cw], ps[:, :cw])
                    ci += 1
                last_pro_act = nc.scalar.mul(negMTi[:, m, :], MTlo[:, m, 0:SHARD], -1.0)

            # idle-DMA duplication of self+mid into MThi.  absorber_pro makes
            # the ACT clock dominate the DVE prologue copies so the DMACopy
            # wait-pruning below (keep ACT only) stays correct.
            dma_mt1 = nc.sync.dma_start(out=MThi[:, :, 0:1024],
                                        in_=MT[:, :, 0:1024])
            dma_mt = nc.sync.dma_start(out=MThi[:, :, 1024:1152],
                                       in_=MT[:, :, 1152:1280])

            prev_mm_last = {}
            prev_dve_last = {}
            prev_exp = {}
            for q in range(npairs):
                lo = q < HALF
                MTq = MTlo if lo else MThi
                off = 0 if lo else 128
                split = SPLIT_LO if lo else SPLIT_HI
                chunks = CHUNKS_LO if lo else CHUNKS_HI
                cols = (2 * q, 2 * q + 1)

                # --- A tiles
                A = {}
                for s in range(4):
                    for ii in range(2):
                        dt_s = fp8 if (s, ii) in FP8_G else bf16
                        A[(s, ii)] = apool.tile([128, 2, W], dt_s,
                                                name=f"A{s}_{ii}", tag=f"A{s}_{ii}",
                                                bufs=2)

                # --- ACT absorbers: slab ownership is single-engine per tile,
                # so the PE-WAR lands on the first fp8 slab and the DVE-WAW of
                # the shared (2,0) tile lands on (2,0,0) -- one wait each, no
                # separate absorbers needed.
                act_markers = []
                if q == HALF:
                    # one absorber per dup-DMA queue sem; the PE-WAR then
                    # lands alone on the first slab
                    for j, d in enumerate((dma_mt1, dma_mt)):
                        mk = nc.scalar.copy(junkA[:, 1 + j:2 + j],
                                            negMTi[0:1, 2, j:j + 1])
                        add_dep_helper(mk.ins, d.ins, sync=True,
                                       reason="ACT waits MThi dup DMA")
                        act_markers.append(mk)

                # --- ACT absdiff slabs: fp8 groups then bf16 (2,0,0) + tail
                act_slabs = [(0, 0, 0), (0, 0, 1), (1, 0, 0), (1, 0, 1), (2, 0, 0)]
                for (s, ii, kt) in act_slabs:
                    t = 2 * s + kt
                    act_i = nc.scalar.activation(
                        A[(s, ii)][:, kt, off:W], MTq[:, t, off:W], Abs,
                        bias=negMTi[:, t, cols[ii]:cols[ii] + 1])
                    for mk in act_markers:
                        add_dep_helper(act_i.ins, mk.ins, sync=False,
                                       reason="order absdiff after ACT absorbers")
                t = 2 * 2 + 1
                act_i = nc.scalar.activation(
                    A[(2, 0)][:, 1, split:W], MTq[:, t, split:W], Abs,
                    bias=negMTi[:, t, cols[0]:cols[0] + 1])
                for mk in act_markers:
                    add_dep_helper(act_i.ins, mk.ins, sync=False,
                                   reason="order absdiff after ACT absorbers")

                # --- DVE absorbers: only the q==HALF MThi-DMA carrier remains
                dve_markers = []
                if q == HALF:
                    for j, d in enumerate((dma_mt1, dma_mt)):
                        mk = nc.vector.tensor_copy(junkD[:, j:j + 1],
                                                   negMTi[0:1, 3, j:j + 1])
                        add_dep_helper(mk.ins, d.ins, sync=True,
                                       reason="DVE waits MThi dup DMA")
                        dve_markers.append(mk)

                # --- DVE absdiff subs; the ACT-shared (2,0) tile goes LAST so
                # its cross-engine tile wait is the instruction's only one.
                dve_subs = []
                for s in (3,):
                    for kt in range(2):
                        dve_subs.append(((s, 0), kt, off, W))
                for s in range(4):
                    for kt in range(2):
                        dve_subs.append(((s, 1), kt, off, W))
                dve_subs.append(((2, 0), 1, off, split))
                first_sub = True
                for (g, kt, c0, c1) in dve_subs:
                    t = 2 * g[0] + kt
                    ts = nc.vector.tensor_scalar(
                        A[g][:, kt, c0:c1], MTq[:, t, c0:c1],
                        negMTi[:, t, cols[g[1]]:cols[g[1]] + 1], None, op0=add_op)
                    for mk in dve_markers:
                        # sync on the first sub propagates the absorber's
                        # vector clock (a nosync edge only orders, leaving a
                        # second redundant sem wait at q==0)
                        add_dep_helper(ts.ins, mk.ins, sync=first_sub,
                                       reason="order after DVE absorbers")
                    first_sub = False

                # --- DVE sign-clear passes: joints, then the (2,0) partial
                for g in ((3, 0), (0, 1), (1, 1), (2, 1), (3, 1)):
                    view = A[g][:, :, off:W].bitcast(u16)
                    nc.vector.tensor_scalar(view, view, mask16, None, op0=band_op)
                view = A[(2, 0)][:, 1, off:split].bitcast(u16)
                dve_last = nc.vector.tensor_scalar(view, view, mask16, None,
                                                   op0=band_op)

                # --- PE: c-contraction into psum [2i x 64b, W]
                ps = mpsum.tile([128, 1536], f32, name="ps", tag="ps")
                if q >= 2:
                    dmy = nc.tensor.matmul(ps[0:32, 0:1], Sbig[:, 0:32],
                                           Sbig[:, 1:2], start=True, stop=True,
                                           skip_group_check=True)
                    add_dep_helper(dmy.ins, prev_mm_last[q - 2].ins,
                                   sync=True, reason="advance PE own-clock for psum WAW")
                mm1 = None
                mm = None
                first_group = True
                for ii in range(2):
                    for (clo, cw) in chunks:
                        out = ps[64 * ii:64 * (ii + 1), clo:clo + cw]
                        if ii == 0:
                            mm = nc.tensor.matmul(
                                out, SelDR[:, :, 64:128],
                                A[(0, 0)][:, :, clo:clo + cw],
                                start=True, stop=False, perf_mode=DR)
                            if mm1 is None:
                                mm1 = mm
                                add_dep_helper(mm1.ins, act_i.ins, sync=True,
                                               reason="PE waits last ACT slab")
                            nc.tensor.matmul(
                                out, SelDR[:, :, 48:112],
                                A[(1, 0)][:, :, clo:clo + cw],
                                start=False, stop=False, perf_mode=DR)
                            for s in (2, 3):
                                for kt in range(2):
                                    t = 2 * s + kt
                                    mm = nc.tensor.matmul(
                                        out, Sbig[:, 56 - 8 * t:120 - 8 * t],
                                        A[(s, 0)][:, kt, clo:clo + cw],
                                        start=False, stop=(s == 3 and kt == 1))
                        else:
                            for s in range(4):
                                for kt in range(2):
                                    t = 2 * s + kt
                                    mm = nc.tensor.matmul(
                                        out, Sbig[:, 56 - 8 * t:120 - 8 * t],
                                        A[(s, 1)][:, kt, clo:clo + cw],
                                        start=(s == 0 and kt == 0),
                                        stop=(s == 3 and kt == 1))
                        if first_group and q >= 1:
                            # column-sums for pair q-1 (expS ready)
                            plo = (q - 1) < HALF
                            eprev = prev_exp[q - 1][1]
                            nc.tensor.matmul(csA[0:64], SelCS, eprev[:, 256:768],
                                             start=(q == 1), stop=(q == npairs),
                                             skip_group_check=True)
                            nc.tensor.matmul(csB[0:64, 0:256], SelCS,
                                             eprev[:, 768:1024],
                                             start=(q == 1), stop=(q == npairs),
                                             skip_group_check=True)
                            if plo:
                                nc.tensor.matmul(csB[0:64, 256:384], SelCS,
                                                 eprev[:, 128:256],
                                                 start=(q == 1), stop=(q == HALF),
                                                 skip_group_check=True)
                                nc.tensor.matmul(csB[0:64, 384:512], SelCS,
                                                 eprev[:, 1024:1152],
                                                 start=(q == 1), stop=(q == HALF),
                                                 skip_group_check=True)
                            else:
                                nc.tensor.matmul(csB[0:64, 256:384], SelCS,
                                                 eprev[:, 1024:1152],
                                                 start=(q == HALF + 1),
                                                 stop=(q == npairs),
                                                 skip_group_check=True)
                        first_group = False

                # --- csSelf evacuation right after its last accumulation
                if q == HALF:
                    cse = nc.scalar.copy(csS[0:64, 768:896], csB[0:64, 256:384])

                # --- exp -> SBUF bf16; accum_out writes the rowsum column
                if q >= 1:
                    absorber_ex = nc.scalar.copy(junkX[:, q:q + 1],
                                                 negMTi[0:1, 2, q:q + 1])
                    add_dep_helper(absorber_ex.ins, prev_exp[q - 1][0].ins,
                                   sync=True, reason="advance ACT own accum clock")
                    add_dep_helper(absorber_ex.ins, act_i.ins, sync=False,
                                   reason="pin after this pair's absdiffs")
                expS = epool.tile([128, W], bf16, name="expS", tag="expS", bufs=2)
                exp_i = nc.scalar.activation(expS[:, off:W], ps[:, off:W], Exp,
                                             scale=-1.0,
                                             accum_out=O_sb[:, q:q + 1])
                prev_exp[q] = (exp_i, expS)
                prev_mm_last[q] = mm
                prev_dve_last[q] = dve_last

            # column-sums of the last pair
            elast = prev_exp[npairs - 1][1]
            nc.tensor.matmul(csA[0:64], SelCS, elast[:, 256:768],
                             start=(npairs == 1), stop=True, skip_group_check=True)
            nc.tensor.matmul(csB[0:64, 0:256], SelCS, elast[:, 768:1024],
                             start=(npairs == 1), stop=True, skip_group_check=True)
            if npairs - 1 < HALF:
                nc.tensor.matmul(csB[0:64, 256:384], SelCS, elast[:, 128:256],
                                 start=(npairs == 1), stop=True,
                                 skip_group_check=True)
                nc.tensor.matmul(csB[0:64, 384:512], SelCS, elast[:, 1024:1152],
                                 start=(npairs == 1), stop=True,
                                 skip_group_check=True)
            else:
                nc.tensor.matmul(csB[0:64, 256:384], SelCS, elast[:, 1024:1152],
                                 start=(npairs == HALF + 1), stop=True,
                                 skip_group_check=True)

            nc.scalar.copy(csS[0:64, 0:512], csA[0:64])
            nc.scalar.copy(csS[0:64, 512:768], csB[0:64, 0:256])
            if npairs > HALF:
                nc.scalar.copy(csS[0:64, 1024:1152], csB[0:64, 256:384])
            else:
                nc.scalar.copy(csS[0:64, 768:896], csB[0:64, 256:384])
            nc.scalar.copy(csS[0:64, 896:1024], csB[0:64, 384:512])
            dma_o = nc.sync.dma_start(out=OUT, in_=OutS)

    # Kernel-tail Drain wait pruning (see baseline): every proc's completion is
    # transitively dominated by the output DMA; keep only its queue sem.
    out_upd = {u.ant_name for u in dma_o.ins.sync_info.on_update}
    for f in nc.m.functions:
        for bb in f.blocks:
            for ins in bb.instructions:
                si = getattr(ins, 'sync_info', None)
                if si is None or len(si.on_wait) <= 1:
                    continue
                if ins.opcode == 'Drain':
                    kept = [w for w in si.on_wait if w.ant_name in out_upd]
                    assert kept, f"drain {ins.name} has no output-queue wait"
                    si.on_wait = kept
                elif ins.opcode == 'DMACopy':
                    kept = [w for w in si.on_wait if 'Activation' in w.ant_name]
                    assert kept, f"dma {ins.name} has no ACT wait"
                    si.on_wait = kept
    return nc


def _host_inputs(x, T):
    bf = ml_dtypes.bfloat16
    xT = np.ascontiguousarray(np.asarray(x, dtype=np.float32).T).astype(bf)   # [512, 2048]
    Tm = np.asarray(T, dtype=np.float32).reshape(A_DIM, BC).astype(bf)

    consts = np.zeros((A_DIM, TOT_W - SB_OFF), dtype=bf)
    # Sbig: [128, 120] with Sb[p, 56 + p//16] = 1 (windows for bf16 matmuls)
    for p in range(128):
        consts[p, 56 + p // 16] = 1
    # SelDR: [128, 2, 128] fp8, sel[p, kt, 64 + 8kt + p//16] = 1; windows
    # [64:128) select s=0, [48:112) select s=1.
    sel = np.zeros((128, 2, 128), dtype=ml_dtypes.float8_e4m3)
    for p in range(128):
        for kt in range(2):
            sel[p, kt, 64 + 8 * kt + p // 16] = 1
    selb = sel.reshape(128, 256).view(np.uint16).view(bf)                      # [128, 128]
    consts[:128, SELDR_OFF - SB_OFF:SELCS_OFF - SB_OFF] = selb
    # SelCS: [128, 64] with selcs[p, p % 64] = 1
    for p in range(128):
        consts[p, SELCS_OFF - SB_OFF + (p % 64)] = 1

    in_maps = []
    for c in range(N_CORES):
        xT_rot = np.roll(xT, -c * SHARD, axis=1)[:, :WIN].copy()
        if c >= 4:
            anti = xT_rot[:, 1024:1280].copy()
            xT_rot[:, 1024:1152] = anti[:, 128:256]
            xT_rot[:, 1152:1280] = anti[:, 0:128]
        XTmc = np.ascontiguousarray(
            np.concatenate([xT_rot, Tm, consts], axis=1))
        in_maps.append({"XTm": XTmc})
    return in_maps


def run(x, T, npairs=PAIRS, trace=False):
    from concourse.bass_utils import run_bass_kernel_spmd

    nc = _CACHE.get(npairs)
    if nc is None:
        nc = _build_nc(npairs)
        _CACHE[npairs] = nc
    in_maps = _host_inputs(x, T)
    res = run_bass_kernel_spmd(nc, in_maps, list(range(N_CORES)), trace=trace)
    parts = []
    for c in range(N_CORES):
        out = np.asarray(res.results[c]["OUT"], dtype=np.float32)
        r = out[:, :PAIRS]                                   # [64ii+b, q]
        parts.append(r.reshape(2, B, PAIRS).transpose(2, 0, 1).reshape(SHARD, B))
    O = np.concatenate(parts, axis=0)
    for c in range(N_CORES):
        cs = np.asarray(res.results[c]["OUT"], dtype=np.float32)[0:B, PAIRS:]
        # common: local cols [256, 1024) -> global rows (256c + L) % N
        idx = (np.arange(256, 1024) + SHARD * c) % N
        O[idx] += cs[:, 0:768].T
        # csSelf: local cols [128, 256) -> own shard rows
        idx = SHARD * c + np.arange(128, 256)
        O[idx] += cs[:, 768:896].T
        # antipodal: partner block rows
        partner = (c + 4) % N_CORES
        if c < 4:
            lo_rows = SHARD * partner + np.arange(0, 128)
            hi_rows = SHARD * partner + np.arange(128, 256)
        else:
            lo_rows = SHARD * partner + np.arange(128, 256)
            hi_rows = SHARD * partner + np.arange(0, 128)
        O[lo_rows] += cs[:, 896:1024].T
        O[hi_rows] += cs[:, 1024:1152].T
    return O, res


def kernel(x, T):
    O, _ = run(x, T)
    return O


# revision 28
# speedup vs baseline: 1.0396x; 1.0167x over previous
"""Trainium2 Bass kernel for MinibatchDiscrimination (symmetric scheme, v2).

Reference computation:
    M = (x @ T.reshape(A, B*C)).reshape(N, B, C)        x:[2048,512] T:[512,64,16]
    O[i, b] = sum_j exp(-sum_c |M[j,b,c] - M[i,b,c]|)    O:[2048,64]

K(i,j) is symmetric; each unordered pair is computed once and credited to both
row-sums.  Core c owns rows R_c = [256c, 256c+256), processed as 128 pairs q
(rows 2q, 2q+1).  Coverage per pair (local window cols):

  - self block  [0, 256):   pairs q < 64 compute all 256 cols; pairs q >= 64
    compute only [128, 256).  The missing orientation (rows [128,256) x cols
    [0,128)) is recovered from the q<64 exps via a column-sum over local cols
    [128, 256) (csSelf).  Intra-block pairs otherwise appear in both
    orientations, so no other self credit is needed.
  - next 3 blocks [256, 1024): full; column-sums credit the partner rows.
  - antipodal block (c+4): 128 of its 256 cols at FULL weight: pairs q<64 get
    one half, q>=64 the complementary half; cores 4-7 use the swapped halves
    (host swaps the two 128-col groups in their input) so that every
    cross-antipodal pair is computed exactly once.  Column-sums (csAntiLo for
    q<64, csAntiHi for q>=64) credit the partner rows.

So window W_q = 1152 for q < 64 and 1024 for q >= 64.

Engine split per pair (production of absdiff slabs A[(s,ii)] = |MT - MT[:,i]|,
bc on partitions as 8 t-blocks x 128, window on free dim):
  - ACT: s=0,1 / ii=0 slabs in fp8 (feed DoubleRow matmuls), (2,0,0) bf16 and
    the tail [SPLIT:] of (2,0,1); also the exp and its accum_out (row-sums).
  - DVE: everything else in bf16 via 2 passes (tensor_scalar add-neg, then
    joint u16 sign-clear).
  - PE: c-contraction into psum [2i x 64b, W], + column-sum matmuls.
MT is kept twice: MT_lo (self+mid+antiLo, 1152) and MT_hi (self+mid+antiHi)
copied via an idle SBUF->SBUF DMA so every slab read is contiguous.

Sync discipline: every engine instruction carries at most ONE sem wait
(walrus limit); cross-engine deps are subsumed through per-pair absorber
instructions (baseline-proven pattern).
"""

import numpy as np
import ml_dtypes

N, A_DIM, B, C = 2048, 512, 64, 16
BC = B * C
N_CORES = 8
SHARD = N // N_CORES          # 256
PAIRS = SHARD // 2            # 128
HALF = PAIRS // 2             # 64 pairs per q-half
W = 1152                      # self + mid + one antipodal half
WIN = 1280                    # prologue j window (both antipodal halves)

XT_W = WIN
TM_OFF = XT_W                 # 1280
SB_OFF = TM_OFF + BC          # 2304
SELDR_OFF = SB_OFF + 120      # 2424
SELCS_OFF = SELDR_OFF + 128   # 2552
TOT_W = SELCS_OFF + 64        # 2616

# ACT-produced slabs: (s,ii) in FP8_G are fp8 (both kt); (2,0,0) is ACT bf16;
# (2,0,1) is split: DVE does [off_q, SPLIT), ACT does [SPLIT, 1152).
FP8_G = ((0, 0), (1, 0))
SPLIT_LO = 886                # q < 64  (window [0, 1152))
SPLIT_HI = 920                # q >= 64 (window [128, 1152))

CHUNKS_LO = ((0, 512), (512, 512), (1024, 128))
CHUNKS_HI = ((128, 384), (512, 512), (1024, 128))

# cs psum layout (2 banks):
#   csA [128,512]  <- eprev[:, 256:768)                       (all pairs)
#   csB [128,512]: [0:256)   <- eprev[:, 768:1024)            (all pairs)
#                  [256:384) <- eprev[:, 128:256)  q<64 csSelf, then reused
#                               as csAntiHi (q>=64, eprev[:, 1024:1152))
#                  [384:512) <- eprev[:, 1024:1152) q<64 csAntiLo
# csSelf is copied to OutS right after pair 64's cs matmuls, then the slot is
# restarted (start=True) as csAntiHi.
CS_OUT = 768 + 128 + 128 + 128  # common 768 | self 128 | antiLo 128 | antiHi 128
OUT_W = PAIRS + CS_OUT          # 1280

_CACHE = {}


def _build_nc(npairs=PAIRS):
    from contextlib import ExitStack
    import concourse.bass as bass
    import concourse.mybir as mybir
    from concourse.tile import TileContext, add_dep_helper

    f32 = mybir.dt.float32
    bf16 = mybir.dt.bfloat16
    fp8 = mybir.dt.float8e4
    u16 = mybir.dt.uint16
    Abs = mybir.ActivationFunctionType.Abs
    Exp = mybir.ActivationFunctionType.Exp
    add_op = mybir.AluOpType.add
    band_op = mybir.AluOpType.bitwise_and
    DR = mybir.MatmulPerfMode.DoubleRow

    nc = bass.Bass("TRN2", target_bir_lowering=False, debug=False)
    XTm = nc.dram_tensor("XTm", [A_DIM, TOT_W], bf16, kind="ExternalInput").ap()
    OUT = nc.dram_tensor("OUT", [128, OUT_W], f32, kind="ExternalOutput").ap()

    with TileContext(nc) as tc, ExitStack() as ctx:
        singles = ctx.enter_context(tc.tile_pool(name="singles", bufs=1))

        XAll = singles.tile([128, 4, TOT_W], bf16, name="XAll", tag="XAll")
        # split by k so prologue k=0 matmuls (and const copies) start after a
        # quarter of the input transfer
        for k in range(4):
            nc.sync.dma_start(out=XAll[:, k], in_=XTm[128 * k:128 * (k + 1), :])

        Sbig = singles.tile([128, 120], bf16, name="Sbig", tag="Sbig")
        nc.scalar.copy(Sbig, XAll[:, 0, SB_OFF:SB_OFF + 120])
        SelDR = singles.tile([128, 2, 128], fp8, name="SelDR", tag="SelDR")
        nc.scalar.copy(SelDR.rearrange("p a b -> p (a b)"),
                       XAll[:, 0, SELDR_OFF:SELDR_OFF + 128].bitcast(fp8))
        SelCS = singles.tile([128, 64], bf16, name="SelCS", tag="SelCS")
        nc.scalar.copy(SelCS, XAll[:, 0, SELCS_OFF:SELCS_OFF + 64])

        MT = singles.tile([128, 8, WIN], bf16, name="MT", tag="MT")
        MTlo = MT[:, :, 0:W]
        MThi = singles.tile([128, 8, W], bf16, name="MThi", tag="MThi")
        negMTi = singles.tile([128, 8, SHARD], f32, name="negMTi", tag="negMTi")
        OutS = singles.tile([128, OUT_W], f32, name="OutS", tag="OutS")
        O_sb = OutS[:, 0:PAIRS]
        csS = OutS[:, PAIRS:]
        mask16 = singles.tile([128, 1], u16, name="mask16", tag="mask16")
        nc.vector.memset(mask16, 0x7FFF)
        junkD = singles.tile([1, PAIRS], f32, name="junkD", tag="junkD")
        junkA = singles.tile([1, PAIRS], f32, name="junkA", tag="junkA")
        junkE = singles.tile([1, PAIRS], f32, name="junkE", tag="junkE")
        junkX = singles.tile([1, PAIRS], f32, name="junkX", tag="junkX")

        with tc.tile_pool(name="cspsum", bufs=1, space="PSUM") as cspool, \
             tc.tile_pool(name="mpsum", bufs=2, space="PSUM") as mpsum, \
             tc.tile_pool(name="apool", bufs=2) as apool, \
             tc.tile_pool(name="epool", bufs=2) as epool:
            csA = cspool.tile([128, 512], f32, name="csA", tag="csA")
            csB = cspool.tile([128, 512], f32, name="csB", tag="csB")

            # ---- prologue: MT = (x @ T)^T in bf16 (psum f32); cols [0:1152)
            # -> MTlo, [1152:1280) -> MThi[..., 1024:1152).  psum->SBUF copies
            # alternate ACT/DVE.  Then one SBUF->SBUF DMA dups [0:1024).
            last_pro_dve = None
            ci = 0
            for m in range(8):
                for (clo, cw) in ((0, 512), (512, 512), (1024, 256)):
                    ps = mpsum.tile([128, 1536], f32, name="ps", tag="ps")
                    for k in range(4):
                        nc.tensor.matmul(
                            ps[:, :cw],
                            XAll[:, k, TM_OFF + 128 * m:TM_OFF + 128 * (m + 1)],
                            XAll[:, k, clo:clo + cw],
                            start=(k == 0), stop=(k == 3),
                        )
                    # all MT copies on ACT: single-engine MT writers keep every
                    # consumer at one sem wait regardless of scheduler sinking
                    nc.scalar.copy(MT[:, m, clo:clo + cw], ps[:, :cw])
                    ci += 1
                last_pro_act = nc.scalar.mul(negMTi[:, m, :], MTlo[:, m, 0:SHARD], -1.0)

            # idle-DMA duplication of self+mid into MThi.  absorber_pro makes
            # the ACT clock dominate the DVE prologue copies so the DMACopy
            # wait-pruning below (keep ACT only) stays correct.
            dma_mt1 = nc.sync.dma_start(out=MThi[:, :, 0:1024],
                                        in_=MT[:, :, 0:1024])
            dma_mt = nc.sync.dma_start(out=MThi[:, :, 1024:1152],
                                       in_=MT[:, :, 1152:1280])

            prev_mm_last = {}
            prev_dve_last = {}
            prev_exp = {}
            for q in range(npairs):
                lo = q < HALF
                MTq = MTlo if lo else MThi
                off = 0 if lo else 128
                split = SPLIT_LO if lo else SPLIT_HI
                chunks = CHUNKS_LO if lo else CHUNKS_HI
                cols = (2 * q, 2 * q + 1)

                # --- A tiles
                A = {}
                for s in range(4):
                    for ii in range(2):
                        dt_s = fp8 if (s, ii) in FP8_G else bf16
                        A[(s, ii)] = apool.tile([128, 2, W], dt_s,
                                                name=f"A{s}_{ii}", tag=f"A{s}_{ii}",
                                                bufs=2)

                # --- ACT absorbers: slab ownership is single-engine per tile,
                # so the PE-WAR lands on the first fp8 slab and the DVE-WAW of
                # the shared (2,0) tile lands on (2,0,0) -- one wait each, no
                # separate absorbers needed.
                act_markers = []
                if q == HALF:
                    # one absorber per dup-DMA queue sem; the PE-WAR then
                    # lands alone on the first slab
                    for j, d in enumerate((dma_mt1, dma_mt)):
                        mk = nc.scalar.copy(junkA[:, 1 + j:2 + j],
                                            negMTi[0:1, 2, j:j + 1])
                        add_dep_helper(mk.ins, d.ins, sync=True,
                                       reason="ACT waits MThi dup DMA")
                        act_markers.append(mk)

                # --- ACT absdiff slabs: fp8 groups then bf16 (2,0,0) + tail
                act_slabs = [(0, 0, 0), (0, 0, 1), (1, 0, 0), (1, 0, 1), (2, 0, 0)]
                for (s, ii, kt) in act_slabs:
                    t = 2 * s + kt
                    act_i = nc.scalar.activation(
                        A[(s, ii)][:, kt, off:W], MTq[:, t, off:W], Abs,
                        bias=negMTi[:, t, cols[ii]:cols[ii] + 1])
                    for mk in act_markers:
                        add_dep_helper(act_i.ins, mk.ins, sync=False,
                                       reason="order absdiff after ACT absorbers")
                t = 2 * 2 + 1
                act_i = nc.scalar.activation(
                    A[(2, 0)][:, 1, split:W], MTq[:, t, split:W], Abs,
                    bias=negMTi[:, t, cols[0]:cols[0] + 1])
                for mk in act_markers:
                    add_dep_helper(act_i.ins, mk.ins, sync=False,
                                   reason="order absdiff after ACT absorbers")

                # --- DVE absorbers: only the q==HALF MThi-DMA carrier remains
                dve_markers = []
                if q == HALF:
                    for j, d in enumerate((dma_mt1, dma_mt)):
                        mk = nc.vector.tensor_copy(junkD[:, j:j + 1],
                                                   negMTi[0:1, 3, j:j + 1])
                        add_dep_helper(mk.ins, d.ins, sync=True,
                                       reason="DVE waits MThi dup DMA")
                        dve_markers.append(mk)

                # --- DVE absdiff subs; the ACT-shared (2,0) tile goes LAST so
                # its cross-engine tile wait is the instruction's only one.
                dve_subs = []
                for s in (3,):
                    for kt in range(2):
                        dve_subs.append(((s, 0), kt, off, W))
                for s in range(4):
                    for kt in range(2):
                        dve_subs.append(((s, 1), kt, off, W))
                dve_subs.append(((2, 0), 1, off, split))
                first_sub = True
                for (g, kt, c0, c1) in dve_subs:
                    t = 2 * g[0] + kt
                    ts = nc.vector.tensor_scalar(
                        A[g][:, kt, c0:c1], MTq[:, t, c0:c1],
                        negMTi[:, t, cols[g[1]]:cols[g[1]] + 1], None, op0=add_op)
                    for mk in dve_markers:
                        # sync on the first sub propagates the absorber's
                        # vector clock (a nosync edge only orders, leaving a
                        # second redundant sem wait at q==0)
                        add_dep_helper(ts.ins, mk.ins, sync=first_sub,
                                       reason="order after DVE absorbers")
                    first_sub = False

                # --- DVE sign-clear passes: joints, then the (2,0) partial
                for g in ((3, 0), (0, 1), (1, 1), (2, 1), (3, 1)):
                    view = A[g][:, :, off:W].bitcast(u16)
                    nc.vector.tensor_scalar(view, view, mask16, None, op0=band_op)
                view = A[(2, 0)][:, 1, off:split].bitcast(u16)
                dve_last = nc.vector.tensor_scalar(view, view, mask16, None,
                                                   op0=band_op)

                # --- PE: c-contraction into psum [2i x 64b, W]
                ps = mpsum.tile([128, 1536], f32, name="ps", tag="ps")
                if q >= 2:
                    dmy = nc.tensor.matmul(ps[0:32, 0:1], Sbig[:, 0:32],
                                           Sbig[:, 1:2], start=True, stop=True,
                                           skip_group_check=True)
                    add_dep_helper(dmy.ins, prev_mm_last[q - 2].ins,
                                   sync=True, reason="advance PE own-clock for psum WAW")
                mm1 = None
                mm = None
                first_group = True
                for ii in range(2):
                    for (clo, cw) in chunks:
                        out = ps[64 * ii:64 * (ii + 1), clo:clo + cw]
                        if ii == 0:
                            mm = nc.tensor.matmul(
                                out, SelDR[:, :, 64:128],
                                A[(0, 0)][:, :, clo:clo + cw],
                                start=True, stop=False, perf_mode=DR)
                            if mm1 is None:
                                mm1 = mm
                                add_dep_helper(mm1.ins, act_i.ins, sync=True,
                                               reason="PE waits last ACT slab")
                            nc.tensor.matmul(
                                out, SelDR[:, :, 48:112],
                                A[(1, 0)][:, :, clo:clo + cw],
                                start=False, stop=False, perf_mode=DR)
                            for s in (2, 3):
                                for kt in range(2):
                                    t = 2 * s + kt
                                    mm = nc.tensor.matmul(
                                        out, Sbig[:, 56 - 8 * t:120 - 8 * t],
                                        A[(s, 0)][:, kt, clo:clo + cw],
                                        start=False, stop=(s == 3 and kt == 1))
                        else:
                            for s in range(4):
                                for kt in range(2):
                                    t = 2 * s + kt
                                    mm = nc.tensor.matmul(
                                        out, Sbig[:, 56 - 8 * t:120 - 8 * t],
                                        A[(s, 1)][:, kt, clo:clo + cw],
                                        start=(s == 0 and kt == 0),
                                        stop=(s == 3 and kt == 1))
                        if first_group and q >= 1:
                            # column-sums for pair q-1 (expS ready)
                            plo = (q - 1) < HALF
                            eprev = prev_exp[q - 1][1]
                            nc.tensor.matmul(csA[0:64], SelCS, eprev[:, 256:768],
                                             start=(q == 1), stop=(q == npairs),
                                             skip_group_check=True)
                            nc.tensor.matmul(csB[0:64, 0:256], SelCS,
                                             eprev[:, 768:1024],
                                             start=(q == 1), stop=(q == npairs),
                                             skip_group_check=True)
                            if plo:
                                nc.tensor.matmul(csB[0:64, 256:384], SelCS,
                                                 eprev[:, 128:256],
                                                 start=(q == 1), stop=(q == HALF),
                                                 skip_group_check=True)
                                nc.tensor.matmul(csB[0:64, 384:512], SelCS,
                                                 eprev[:, 1024:1152],
                                                 start=(q == 1), stop=(q == HALF),
                                                 skip_group_check=True)
                            else:
                                nc.tensor.matmul(csB[0:64, 256:384], SelCS,
                                                 eprev[:, 1024:1152],
                                                 start=(q == HALF + 1),
                                                 stop=(q == npairs),
                                                 skip_group_check=True)
                        first_group = False

                # --- csSelf evacuation right after its last accumulation
                if q == HALF:
                    cse = nc.scalar.copy(csS[0:64, 768:896], csB[0:64, 256:384])

                # --- exp -> SBUF bf16; accum_out writes the rowsum column
                if q >= 1:
                    absorber_ex = nc.scalar.copy(junkX[:, q:q + 1],
                                                 negMTi[0:1, 2, q:q + 1])
                    add_dep_helper(absorber_ex.ins, prev_exp[q - 1][0].ins,
                                   sync=True, reason="advance ACT own accum clock")
                    add_dep_helper(absorber_ex.ins, act_i.ins, sync=False,
                                   reason="pin after this pair's absdiffs")
                expS = epool.tile([128, W], bf16, name="expS", tag="expS", bufs=2)
                exp_i = nc.scalar.activation(expS[:, off:W], ps[:, off:W], Exp,
                                             scale=-1.0,
                                             accum_out=O_sb[:, q:q + 1])
                prev_exp[q] = (exp_i, expS)
                prev_mm_last[q] = mm
                prev_dve_last[q] = dve_last

            # column-sums of the last pair
            elast = prev_exp[npairs - 1][1]
            nc.tensor.matmul(csA[0:64], SelCS, elast[:, 256:768],
                             start=(npairs == 1), stop=True, skip_group_check=True)
            nc.tensor.matmul(csB[0:64, 0:256], SelCS, elast[:, 768:1024],
                             start=(npairs == 1), stop=True, skip_group_check=True)
            if npairs - 1 < HALF:
                nc.tensor.matmul(csB[0:64, 256:384], SelCS, elast[:, 128:256],
                                 start=(npairs == 1), stop=True,
                                 skip_group_check=True)
                nc.tensor.matmul(csB[0:64, 384:512], SelCS, elast[:, 1024:1152],
                                 start=(npairs == 1), stop=True,
                                 skip_group_check=True)
            else:
                nc.tensor.matmul(csB[0:64, 256:384], SelCS, elast[:, 1024:1152],
                                 start=(npairs == HALF + 1), stop=True,
                                 skip_group_check=True)

            nc.scalar.copy(csS[0:64, 0:512], csA[0:64])
            nc.scalar.copy(csS[0:64, 512:768], csB[0:64, 0:256])
            if npairs > HALF:
                nc.scalar.copy(csS[0:64, 1024:1152], csB[0:64, 256:384])
            else:
                nc.scalar.copy(csS[0:64, 768:896], csB[0:64, 256:384])
            nc.scalar.copy(csS[0:64, 896:1024], csB[0:64, 384:512])
            dma_o = nc.sync.dma_start(out=OUT, in_=OutS)

    # Kernel-tail Drain wait pruning (see baseline): every proc's completion is
    # transitively dominated by the output DMA; keep only its queue sem.
    out_upd = {u.ant_name for u in dma_o.ins.sync_info.on_update}
    for f in nc.m.functions:
        for bb in f.blocks:
            for ins in bb.instructions:
                si = getattr(ins, 'sync_info', None)
                if si is None or len(si.on_wait) <= 1:
                    continue
                if ins.opcode == 'Drain':
                    kept = [w for w in si.on_wait if w.ant_name in out_upd]
                    assert kept, f"drain {ins.name} has no output-queue wait"
                    si.on_wait = kept
                elif ins.opcode == 'DMACopy':
                    kept = [w for w in si.on_wait if 'Activation' in w.ant_name]
                    assert kept, f"dma {ins.name} has no ACT wait"
                    si.on_wait = kept
    return nc


def _host_inputs(x, T):
    bf = ml_dtypes.bfloat16
    xT = np.ascontiguousarray(np.asarray(x, dtype=np.float32).T).astype(bf)   # [512, 2048]
    Tm = np.asarray(T, dtype=np.float32).reshape(A_DIM, BC).astype(bf)

    consts = np.zeros((A_DIM, TOT_W - SB_OFF), dtype=bf)
    # Sbig: [128, 120] with Sb[p, 56 + p//16] = 1 (windows for bf16 matmuls)
    for p in range(128):
        consts[p, 56 + p // 16] = 1
    # SelDR: [128, 2, 128] fp8, sel[p, kt, 64 + 8kt + p//16] = 1; windows
    # [64:128) select s=0, [48:112) select s=1.
    sel = np.zeros((128, 2, 128), dtype=ml_dtypes.float8_e4m3)
    for p in range(128):
        for kt in range(2):
            sel[p, kt, 64 + 8 * kt + p // 16] = 1
    selb = sel.reshape(128, 256).view(np.uint16).view(bf)                      # [128, 128]
    consts[:128, SELDR_OFF - SB_OFF:SELCS_OFF - SB_OFF] = selb
    # SelCS: [128, 64] with selcs[p, p % 64] = 1
    for p in range(128):
        consts[p, SELCS_OFF - SB_OFF + (p % 64)] = 1

    in_maps = []
    for c in range(N_CORES):
        xT_rot = np.roll(xT, -c * SHARD, axis=1)[:, :WIN].copy()
        if c >= 4:
            anti = xT_rot[:, 1024:1280].copy()
            xT_rot[:, 1024:1152] = anti[:, 128:256]
            xT_rot[:, 1152:1280] = anti[:, 0:128]
        XTmc = np.ascontiguousarray(
            np.concatenate([xT_rot, Tm, consts], axis=1))
        in_maps.append({"XTm": XTmc})
    return in_maps


def run(x, T, npairs=PAIRS, trace=False):
    from concourse.bass_utils import run_bass_kernel_spmd

    nc = _CACHE.get(npairs)
    if nc is None:
        nc = _build_nc(npairs)
        _CACHE[npairs] = nc
    in_maps = _host_inputs(x, T)
    res = run_bass_kernel_spmd(nc, in_maps, list(range(N_CORES)), trace=trace)
    parts = []
    for c in range(N_CORES):
        out = np.asarray(res.results[c]["OUT"], dtype=np.float32)
        r = out[:, :PAIRS]                                   # [64ii+b, q]
        parts.append(r.reshape(2, B, PAIRS).transpose(2, 0, 1).reshape(SHARD, B))
    O = np.concatenate(parts, axis=0)
    for c in range(N_CORES):
        cs = np.asarray(res.results[c]["OUT"], dtype=np.float32)[0:B, PAIRS:]
        # common: local cols [256, 1024) -> global rows (256c + L) % N
        idx = (np.arange(256, 1024) + SHARD * c) % N
        O[idx] += cs[:, 0:768].T
        # csSelf: local cols [128, 256) -> own shard rows
        idx = SHARD * c + np.arange(128, 256)
        O[idx] += cs[:, 768:896].T
        # antipodal: partner block rows
        partner = (c + 4) % N_CORES
        if c < 4:
            lo_rows = SHARD * partner + np.arange(0, 128)
            hi_rows = SHARD * partner + np.arange(128, 256)
        else:
            lo_rows = SHARD * partner + np.arange(128, 256)
            hi_rows = SHARD * partner + np.arange(0, 128)
        O[lo_rows] += cs[:, 896:1024].T
        O[hi_rows] += cs[:, 1024:1152].T
    return O, res


def kernel(x, T):
    O, _ = run(x, T)
    return O


# revision 31
# speedup vs baseline: 1.0532x; 1.0131x over previous
"""Trainium2 Bass kernel for MinibatchDiscrimination (symmetric scheme, v2).

Reference computation:
    M = (x @ T.reshape(A, B*C)).reshape(N, B, C)        x:[2048,512] T:[512,64,16]
    O[i, b] = sum_j exp(-sum_c |M[j,b,c] - M[i,b,c]|)    O:[2048,64]

K(i,j) is symmetric; each unordered pair is computed once and credited to both
row-sums.  Core c owns rows R_c = [256c, 256c+256), processed as 128 pairs q
(rows 2q, 2q+1).  Coverage per pair (local window cols):

  - self block  [0, 256):   pairs q < 64 compute all 256 cols; pairs q >= 64
    compute only [128, 256).  The missing orientation (rows [128,256) x cols
    [0,128)) is recovered from the q<64 exps via a column-sum over local cols
    [128, 256) (csSelf).  Intra-block pairs otherwise appear in both
    orientations, so no other self credit is needed.
  - next 3 blocks [256, 1024): full; column-sums credit the partner rows.
  - antipodal block (c+4): 128 of its 256 cols at FULL weight: pairs q<64 get
    one half, q>=64 the complementary half; cores 4-7 use the swapped halves
    (host swaps the two 128-col groups in their input) so that every
    cross-antipodal pair is computed exactly once.  Column-sums (csAntiLo for
    q<64, csAntiHi for q>=64) credit the partner rows.

So window W_q = 1152 for q < 64 and 1024 for q >= 64.

Engine split per pair (production of absdiff slabs A[(s,ii)] = |MT - MT[:,i]|,
bc on partitions as 8 t-blocks x 128, window on free dim):
  - ACT: s=0,1 / ii=0 slabs in fp8 (feed DoubleRow matmuls), (2,0,0) bf16 and
    the tail [SPLIT:] of (2,0,1); also the exp and its accum_out (row-sums).
  - DVE: everything else in bf16 via 2 passes (tensor_scalar add-neg, then
    joint u16 sign-clear).
  - PE: c-contraction into psum [2i x 64b, W], + column-sum matmuls.
MT is kept twice: MT_lo (self+mid+antiLo, 1152) and MT_hi (self+mid+antiHi)
copied via an idle SBUF->SBUF DMA so every slab read is contiguous.

Sync discipline: every engine instruction carries at most ONE sem wait
(walrus limit); cross-engine deps are subsumed through per-pair absorber
instructions (baseline-proven pattern).
"""

import numpy as np
import ml_dtypes

N, A_DIM, B, C = 2048, 512, 64, 16
BC = B * C
N_CORES = 8
SHARD = N // N_CORES          # 256
PAIRS = SHARD // 2            # 128
HALF = PAIRS // 2             # 64 pairs per q-half
W = 1152                      # self + mid + one antipodal half
WIN = 1280                    # prologue j window (both antipodal halves)

XT_W = WIN
TM_OFF = XT_W                 # 1280
SB_OFF = TM_OFF + BC          # 2304
SELDR_OFF = SB_OFF + 120      # 2424
SELCS_OFF = SELDR_OFF + 128   # 2552
TOT_W = SELCS_OFF + 64        # 2616

# ACT-produced slabs: (s,ii) in FP8_G are fp8 (both kt); (2,0,0) is ACT bf16;
# (2,0,1) is split: DVE does [off_q, SPLIT), ACT does [SPLIT, 1152).
FP8_G = ((0, 0), (1, 0))
SPLIT_LO = 1006               # q < 64  (window [0, 1152))
SPLIT_HI = 1040               # q >= 64 (window [128, 1152))

CHUNKS_LO = ((0, 512), (512, 512), (1024, 128))
CHUNKS_HI = ((128, 384), (512, 512), (1024, 128))

# cs psum layout (2 banks):
#   csA [128,512]  <- eprev[:, 256:768)                       (all pairs)
#   csB [128,512]: [0:256)   <- eprev[:, 768:1024)            (all pairs)
#                  [256:384) <- eprev[:, 128:256)  q<64 csSelf, then reused
#                               as csAntiHi (q>=64, eprev[:, 1024:1152))
#                  [384:512) <- eprev[:, 1024:1152) q<64 csAntiLo
# csSelf is copied to OutS right after pair 64's cs matmuls, then the slot is
# restarted (start=True) as csAntiHi.
CS_OUT = 768 + 128 + 128 + 128  # common 768 | self 128 | antiLo 128 | antiHi 128
OUT_W = PAIRS + CS_OUT          # 1280

_CACHE = {}


def _build_nc(npairs=PAIRS):
    from contextlib import ExitStack
    import concourse.bass as bass
    import concourse.mybir as mybir
    from concourse.tile import TileContext, add_dep_helper

    f32 = mybir.dt.float32
    bf16 = mybir.dt.bfloat16
    fp8 = mybir.dt.float8e4
    u16 = mybir.dt.uint16
    Abs = mybir.ActivationFunctionType.Abs
    Exp = mybir.ActivationFunctionType.Exp
    add_op = mybir.AluOpType.add
    band_op = mybir.AluOpType.bitwise_and
    DR = mybir.MatmulPerfMode.DoubleRow

    nc = bass.Bass("TRN2", target_bir_lowering=False, debug=False)
    XTm = nc.dram_tensor("XTm", [A_DIM, TOT_W], bf16, kind="ExternalInput").ap()
    OUT = nc.dram_tensor("OUT", [128, OUT_W], f32, kind="ExternalOutput").ap()

    with TileContext(nc) as tc, ExitStack() as ctx:
        singles = ctx.enter_context(tc.tile_pool(name="singles", bufs=1))

        XAll = singles.tile([128, 4, TOT_W], bf16, name="XAll", tag="XAll")
        # split by k so prologue k=0 matmuls (and const copies) start after a
        # quarter of the input transfer
        for k in range(4):
            nc.sync.dma_start(out=XAll[:, k], in_=XTm[128 * k:128 * (k + 1), :])

        Sbig = singles.tile([128, 120], bf16, name="Sbig", tag="Sbig")
        nc.scalar.copy(Sbig, XAll[:, 0, SB_OFF:SB_OFF + 120])
        SelDR = singles.tile([128, 2, 128], fp8, name="SelDR", tag="SelDR")
        nc.scalar.copy(SelDR.rearrange("p a b -> p (a b)"),
                       XAll[:, 0, SELDR_OFF:SELDR_OFF + 128].bitcast(fp8))
        SelCS = singles.tile([128, 64], bf16, name="SelCS", tag="SelCS")
        nc.scalar.copy(SelCS, XAll[:, 0, SELCS_OFF:SELCS_OFF + 64])

        MT = singles.tile([128, 8, WIN], bf16, name="MT", tag="MT")
        MTlo = MT[:, :, 0:W]
        MThi = singles.tile([128, 8, W], bf16, name="MThi", tag="MThi")
        negMTi = singles.tile([128, 8, SHARD], f32, name="negMTi", tag="negMTi")
        OutS = singles.tile([128, OUT_W], f32, name="OutS", tag="OutS")
        O_sb = OutS[:, 0:PAIRS]
        csS = OutS[:, PAIRS:]
        mask16 = singles.tile([128, 1], u16, name="mask16", tag="mask16")
        nc.vector.memset(mask16, 0x7FFF)
        junkD = singles.tile([1, PAIRS], f32, name="junkD", tag="junkD")
        junkA = singles.tile([1, PAIRS], f32, name="junkA", tag="junkA")
        junkE = singles.tile([1, PAIRS], f32, name="junkE", tag="junkE")
        junkX = singles.tile([1, PAIRS], f32, name="junkX", tag="junkX")

        with tc.tile_pool(name="cspsum", bufs=1, space="PSUM") as cspool, \
             tc.tile_pool(name="mpsum", bufs=2, space="PSUM") as mpsum, \
             tc.tile_pool(name="apool", bufs=2) as apool, \
             tc.tile_pool(name="epool", bufs=2) as epool:
            csA = cspool.tile([128, 512], f32, name="csA", tag="csA")
            csB = cspool.tile([128, 512], f32, name="csB", tag="csB")

            # ---- prologue: MT = (x @ T)^T in bf16 (psum f32); cols [0:1152)
            # -> MTlo, [1152:1280) -> MThi[..., 1024:1152).  psum->SBUF copies
            # alternate ACT/DVE.  Then one SBUF->SBUF DMA dups [0:1024).
            last_pro_dve = None
            ci = 0
            for m in range(8):
                for (clo, cw) in ((0, 512), (512, 512), (1024, 256)):
                    ps = mpsum.tile([128, 1536], f32, name="ps", tag="ps")
                    for k in range(4):
                        nc.tensor.matmul(
                            ps[:, :cw],
                            XAll[:, k, TM_OFF + 128 * m:TM_OFF + 128 * (m + 1)],
                            XAll[:, k, clo:clo + cw],
                            start=(k == 0), stop=(k == 3),
                        )
                    # all MT copies on ACT: single-engine MT writers keep every
                    # consumer at one sem wait regardless of scheduler sinking
                    nc.scalar.copy(MT[:, m, clo:clo + cw], ps[:, :cw])
                    ci += 1
                last_pro_act = nc.scalar.mul(negMTi[:, m, :], MTlo[:, m, 0:SHARD], -1.0)

            # idle-DMA duplication of self+mid into MThi.  absorber_pro makes
            # the ACT clock dominate the DVE prologue copies so the DMACopy
            # wait-pruning below (keep ACT only) stays correct.
            dma_mt1 = nc.sync.dma_start(out=MThi[:, :, 0:1024],
                                        in_=MT[:, :, 0:1024])
            dma_mt = nc.sync.dma_start(out=MThi[:, :, 1024:1152],
                                       in_=MT[:, :, 1152:1280])

            prev_mm_last = {}
            prev_dve_last = {}
            prev_exp = {}
            for q in range(npairs):
                lo = q < HALF
                MTq = MTlo if lo else MThi
                off = 0 if lo else 128
                split = SPLIT_LO if lo else SPLIT_HI
                chunks = CHUNKS_LO if lo else CHUNKS_HI
                cols = (2 * q, 2 * q + 1)

                # --- A tiles; all ii=1 slabs share one tile so their u16
                # sign-clear is a single DVE instruction
                A = {}
                for s in range(4):
                    dt_s = fp8 if (s, 0) in FP8_G else bf16
                    A[(s, 0)] = apool.tile([128, 2, W], dt_s,
                                           name=f"A{s}_0", tag=f"A{s}_0",
                                           bufs=2)
                A1 = apool.tile([128, 4, 2, W], bf16, name="A1", tag="A1", bufs=2)
                for s in range(4):
                    A[(s, 1)] = A1[:, s]

                # --- ACT absorbers: slab ownership is single-engine per tile,
                # so the PE-WAR lands on the first fp8 slab and the DVE-WAW of
                # the shared (2,0) tile lands on (2,0,0) -- one wait each, no
                # separate absorbers needed.
                act_markers = []
                if q == HALF:
                    # one absorber per dup-DMA queue sem; the PE-WAR then
                    # lands alone on the first slab
                    for j, d in enumerate((dma_mt1, dma_mt)):
                        mk = nc.scalar.copy(junkA[:, 1 + j:2 + j],
                                            negMTi[0:1, 2, j:j + 1])
                        add_dep_helper(mk.ins, d.ins, sync=True,
                                       reason="ACT waits MThi dup DMA")
                        act_markers.append(mk)

                # --- ACT absdiff slabs: fp8 groups then bf16 (2,0,0) + tail
                act_slabs = [(0, 0, 0), (0, 0, 1), (1, 0, 0), (1, 0, 1), (2, 0, 0)]
                for (s, ii, kt) in act_slabs:
                    t = 2 * s + kt
                    act_i = nc.scalar.activation(
                        A[(s, ii)][:, kt, off:W], MTq[:, t, off:W], Abs,
                        bias=negMTi[:, t, cols[ii]:cols[ii] + 1])
                    for mk in act_markers:
                        add_dep_helper(act_i.ins, mk.ins, sync=False,
                                       reason="order absdiff after ACT absorbers")
                t = 2 * 2 + 1
                act_i = nc.scalar.activation(
                    A[(2, 0)][:, 1, split:W], MTq[:, t, split:W], Abs,
                    bias=negMTi[:, t, cols[0]:cols[0] + 1])
                for mk in act_markers:
                    add_dep_helper(act_i.ins, mk.ins, sync=False,
                                   reason="order absdiff after ACT absorbers")

                # --- DVE absorbers: only the q==HALF MThi-DMA carrier remains
                dve_markers = []
                if q == HALF:
                    for j, d in enumerate((dma_mt1, dma_mt)):
                        mk = nc.vector.tensor_copy(junkD[:, j:j + 1],
                                                   negMTi[0:1, 3, j:j + 1])
                        add_dep_helper(mk.ins, d.ins, sync=True,
                                       reason="DVE waits MThi dup DMA")
                        dve_markers.append(mk)

                # --- DVE absdiff subs; the ACT-shared (2,0) tile goes LAST so
                # its cross-engine tile wait is the instruction's only one.
                dve_subs = []
                for s in (3,):
                    for kt in range(2):
                        dve_subs.append(((s, 0), kt, off, W))
                for s in range(4):
                    for kt in range(2):
                        dve_subs.append(((s, 1), kt, off, W))
                dve_subs.append(((2, 0), 1, off, split))
                first_sub = True
                for (g, kt, c0, c1) in dve_subs:
                    t = 2 * g[0] + kt
                    ts = nc.vector.tensor_scalar(
                        A[g][:, kt, c0:c1], MTq[:, t, c0:c1],
                        negMTi[:, t, cols[g[1]]:cols[g[1]] + 1], None, op0=add_op)
                    for mk in dve_markers:
                        # sync on the first sub propagates the absorber's
                        # vector clock (a nosync edge only orders, leaving a
                        # second redundant sem wait at q==0)
                        add_dep_helper(ts.ins, mk.ins, sync=first_sub,
                                       reason="order after DVE absorbers")
                    first_sub = False

                # --- DVE sign-clear passes: (3,0) joint, one merged ii=1
                # pass over the shared A1 tile, then the (2,0) partial
                view = A[(3, 0)][:, :, off:W].bitcast(u16)
                nc.vector.tensor_scalar(view, view, mask16, None, op0=band_op)
                view = A1[:, :, :, off:W].bitcast(u16)
                nc.vector.tensor_scalar(view, view, mask16, None, op0=band_op)
                view = A[(2, 0)][:, 1, off:split].bitcast(u16)
                dve_last = nc.vector.tensor_scalar(view, view, mask16, None,
                                                   op0=band_op)

                # --- PE: c-contraction into psum [2i x 64b, W]
                ps = mpsum.tile([128, 1536], f32, name="ps", tag="ps")
                if q >= 2:
                    dmy = nc.tensor.matmul(ps[0:32, 0:1], Sbig[:, 0:32],
                                           Sbig[:, 1:2], start=True, stop=True,
                                           skip_group_check=True)
                    add_dep_helper(dmy.ins, prev_mm_last[q - 2].ins,
                                   sync=True, reason="advance PE own-clock for psum WAW")
                mm1 = None
                mm = None
                first_group = True
                for ii in range(2):
                    for (clo, cw) in chunks:
                        out = ps[64 * ii:64 * (ii + 1), clo:clo + cw]
                        if ii == 0:
                            mm = nc.tensor.matmul(
                                out, SelDR[:, :, 64:128],
                                A[(0, 0)][:, :, clo:clo + cw],
                                start=True, stop=False, perf_mode=DR)
                            if mm1 is None:
                                mm1 = mm
                                add_dep_helper(mm1.ins, act_i.ins, sync=True,
                                               reason="PE waits last ACT slab")
                            nc.tensor.matmul(
                                out, SelDR[:, :, 48:112],
                                A[(1, 0)][:, :, clo:clo + cw],
                                start=False, stop=False, perf_mode=DR)
                            for s in (2, 3):
                                for kt in range(2):
                                    t = 2 * s + kt
                                    mm = nc.tensor.matmul(
                                        out, Sbig[:, 56 - 8 * t:120 - 8 * t],
                                        A[(s, 0)][:, kt, clo:clo + cw],
                                        start=False, stop=(s == 3 and kt == 1))
                        else:
                            for s in range(4):
                                for kt in range(2):
                                    t = 2 * s + kt
                                    mm = nc.tensor.matmul(
                                        out, Sbig[:, 56 - 8 * t:120 - 8 * t],
                                        A[(s, 1)][:, kt, clo:clo + cw],
                                        start=(s == 0 and kt == 0),
                                        stop=(s == 3 and kt == 1))
                        if first_group and q >= 1:
                            # column-sums for pair q-1 (expS ready)
                            plo = (q - 1) < HALF
                            eprev = prev_exp[q - 1][1]
                            nc.tensor.matmul(csA[0:64], SelCS, eprev[:, 256:768],
                                             start=(q == 1), stop=(q == npairs),
                                             skip_group_check=True)
                            nc.tensor.matmul(csB[0:64, 0:256], SelCS,
                                             eprev[:, 768:1024],
                                             start=(q == 1), stop=(q == npairs),
                                             skip_group_check=True)
                            if plo:
                                nc.tensor.matmul(csB[0:64, 256:384], SelCS,
                                                 eprev[:, 128:256],
                                                 start=(q == 1), stop=(q == HALF),
                                                 skip_group_check=True)
                                nc.tensor.matmul(csB[0:64, 384:512], SelCS,
                                                 eprev[:, 1024:1152],
                                                 start=(q == 1), stop=(q == HALF),
                                                 skip_group_check=True)
                            else:
                                nc.tensor.matmul(csB[0:64, 256:384], SelCS,
                                                 eprev[:, 1024:1152],
                                                 start=(q == HALF + 1),
                                                 stop=(q == npairs),
                                                 skip_group_check=True)
                        first_group = False

                # --- csSelf evacuation right after its last accumulation
                if q == HALF:
                    cse = nc.scalar.copy(csS[0:64, 768:896], csB[0:64, 256:384])

                # --- exp -> SBUF bf16; accum_out writes the rowsum column
                if q >= 1:
                    absorber_ex = nc.scalar.copy(junkX[:, q:q + 1],
                                                 negMTi[0:1, 2, q:q + 1])
                    add_dep_helper(absorber_ex.ins, prev_exp[q - 1][0].ins,
                                   sync=True, reason="advance ACT own accum clock")
                    add_dep_helper(absorber_ex.ins, act_i.ins, sync=False,
                                   reason="pin after this pair's absdiffs")
                expS = epool.tile([128, W], bf16, name="expS", tag="expS", bufs=2)
                exp_i = nc.scalar.activation(expS[:, off:W], ps[:, off:W], Exp,
                                             scale=-1.0,
                                             accum_out=O_sb[:, q:q + 1])
                prev_exp[q] = (exp_i, expS)
                prev_mm_last[q] = mm
                prev_dve_last[q] = dve_last

            # column-sums of the last pair
            elast = prev_exp[npairs - 1][1]
            nc.tensor.matmul(csA[0:64], SelCS, elast[:, 256:768],
                             start=(npairs == 1), stop=True, skip_group_check=True)
            nc.tensor.matmul(csB[0:64, 0:256], SelCS, elast[:, 768:1024],
                             start=(npairs == 1), stop=True, skip_group_check=True)
            if npairs - 1 < HALF:
                nc.tensor.matmul(csB[0:64, 256:384], SelCS, elast[:, 128:256],
                                 start=(npairs == 1), stop=True,
                                 skip_group_check=True)
                nc.tensor.matmul(csB[0:64, 384:512], SelCS, elast[:, 1024:1152],
                                 start=(npairs == 1), stop=True,
                                 skip_group_check=True)
            else:
                nc.tensor.matmul(csB[0:64, 256:384], SelCS, elast[:, 1024:1152],
                                 start=(npairs == HALF + 1), stop=True,
                                 skip_group_check=True)

            nc.scalar.copy(csS[0:64, 0:512], csA[0:64])
            nc.scalar.copy(csS[0:64, 512:768], csB[0:64, 0:256])
            if npairs > HALF:
                nc.scalar.copy(csS[0:64, 1024:1152], csB[0:64, 256:384])
            else:
                nc.scalar.copy(csS[0:64, 768:896], csB[0:64, 256:384])
            nc.scalar.copy(csS[0:64, 896:1024], csB[0:64, 384:512])
            dma_o = nc.sync.dma_start(out=OUT, in_=OutS)

    # Kernel-tail Drain wait pruning (see baseline): every proc's completion is
    # transitively dominated by the output DMA; keep only its queue sem.
    out_upd = {u.ant_name for u in dma_o.ins.sync_info.on_update}
    for f in nc.m.functions:
        for bb in f.blocks:
            for ins in bb.instructions:
                si = getattr(ins, 'sync_info', None)
                if si is None or len(si.on_wait) <= 1:
                    continue
                if ins.opcode == 'Drain':
                    kept = [w for w in si.on_wait if w.ant_name in out_upd]
                    assert kept, f"drain {ins.name} has no output-queue wait"
                    si.on_wait = kept
                elif ins.opcode == 'DMACopy':
                    kept = [w for w in si.on_wait if 'Activation' in w.ant_name]
                    assert kept, f"dma {ins.name} has no ACT wait"
                    si.on_wait = kept
    return nc


def _host_inputs(x, T):
    bf = ml_dtypes.bfloat16
    xT = np.ascontiguousarray(np.asarray(x, dtype=np.float32).T).astype(bf)   # [512, 2048]
    Tm = np.asarray(T, dtype=np.float32).reshape(A_DIM, BC).astype(bf)

    consts = np.zeros((A_DIM, TOT_W - SB_OFF), dtype=bf)
    # Sbig: [128, 120] with Sb[p, 56 + p//16] = 1 (windows for bf16 matmuls)
    for p in range(128):
        consts[p, 56 + p // 16] = 1
    # SelDR: [128, 2, 128] fp8, sel[p, kt, 64 + 8kt + p//16] = 1; windows
    # [64:128) select s=0, [48:112) select s=1.
    sel = np.zeros((128, 2, 128), dtype=ml_dtypes.float8_e4m3)
    for p in range(128):
        for kt in range(2):
            sel[p, kt, 64 + 8 * kt + p // 16] = 1
    selb = sel.reshape(128, 256).view(np.uint16).view(bf)                      # [128, 128]
    consts[:128, SELDR_OFF - SB_OFF:SELCS_OFF - SB_OFF] = selb
    # SelCS: [128, 64] with selcs[p, p % 64] = 1
    for p in range(128):
        consts[p, SELCS_OFF - SB_OFF + (p % 64)] = 1

    in_maps = []
    for c in range(N_CORES):
        xT_rot = np.roll(xT, -c * SHARD, axis=1)[:, :WIN].copy()
        if c >= 4:
            anti = xT_rot[:, 1024:1280].copy()
            xT_rot[:, 1024:1152] = anti[:, 128:256]
            xT_rot[:, 1152:1280] = anti[:, 0:128]
        XTmc = np.ascontiguousarray(
            np.concatenate([xT_rot, Tm, consts], axis=1))
        in_maps.append({"XTm": XTmc})
    return in_maps


def run(x, T, npairs=PAIRS, trace=False):
    from concourse.bass_utils import run_bass_kernel_spmd

    nc = _CACHE.get(npairs)
    if nc is None:
        nc = _build_nc(npairs)
        _CACHE[npairs] = nc
    in_maps = _host_inputs(x, T)
    res = run_bass_kernel_spmd(nc, in_maps, list(range(N_CORES)), trace=trace)
    parts = []
    for c in range(N_CORES):
        out = np.asarray(res.results[c]["OUT"], dtype=np.float32)
        r = out[:, :PAIRS]                                   # [64ii+b, q]
        parts.append(r.reshape(2, B, PAIRS).transpose(2, 0, 1).reshape(SHARD, B))
    O = np.concatenate(parts, axis=0)
    for c in range(N_CORES):
        cs = np.asarray(res.results[c]["OUT"], dtype=np.float32)[0:B, PAIRS:]
        # common: local cols [256, 1024) -> global rows (256c + L) % N
        idx = (np.arange(256, 1024) + SHARD * c) % N
        O[idx] += cs[:, 0:768].T
        # csSelf: local cols [128, 256) -> own shard rows
        idx = SHARD * c + np.arange(128, 256)
        O[idx] += cs[:, 768:896].T
        # antipodal: partner block rows
        partner = (c + 4) % N_CORES
        if c < 4:
            lo_rows = SHARD * partner + np.arange(0, 128)
            hi_rows = SHARD * partner + np.arange(128, 256)
        else:
            lo_rows = SHARD * partner + np.arange(128, 256)
            hi_rows = SHARD * partner + np.arange(0, 128)
        O[lo_rows] += cs[:, 896:1024].T
        O[hi_rows] += cs[:, 1024:1152].T
    return O, res


def kernel(x, T):
    O, _ = run(x, T)
    return O
